# revision 37
# baseline (speedup 1.0000x reference)
"""Trainium2 Bass kernel for nn_ProbAttention (sparse attention / Informer ProbSparse).

Strategy (8 NeuronCores, no collectives):
  core c -> (batch b = c//2, half h = c%2).
  Both cores of a pair compute the full attention/context for their batch;
  the big Wfin product is column-sharded across the pair (n-halves).
  Host sums the partial class scores and adds bfin.

Device pipeline per core (one batch):
  1. K^T, Q^T = W @ X^T (f32r) from packed consts; V(+ones cols) in f32r.
  2. QK = Q @ K^T with the sample mask folded in on PE (ident @ am);
     per tile, DVE gives max (tensor_reduce) and the sampled sum
     (scalar_tensor_tensor vs the count matrix) -> M = max - sum.
     No psX / X@W2 matmuls: the sum comes straight from the QK PSUM.
  3. Rank-based top-140: rank[q] = #{j: M[j] > M[q]} via 8 DVE is_gt
     passes. The rank doubles as the compaction position, so the
     one-hot gather masks e2[qt] = (iota_row == rank_col) come straight
     from the rank columns -- no gpsimd compaction, no index round trips.
     The scatter masks er = (rank_bcast_row == partition_iota) use one
     DRAM relayout of rank that overlaps phase E.
  4. One-hot gather of X_red -> Q_red^T = Wq^T-chunks @ X_red^T (f16);
     scores^T = K @ Q_red^T; exp on ACT; attn@V with fused ones columns
     for the softmax denominators; context via rank-141 scatter matmul
     (+ Wadd residual + vmean fill + badd).
  5. Final 20 half-dots (10 classes x 2 d-halves) fused DVE multiply-reduce
     against the core's Wfin half, partition-reduced by a ones-matmul.

kernel(**inputs) is self-contained: host packs all constants into a few
contiguous DRAM blocks in the exact SBUF layout, and sums partial scores.
"""

import math
import sys

import numpy as np

sys.path.insert(0, "/opt/trn_rl_repo")

import concourse.bass as bass  # noqa: E402
import concourse.bacc as bacc  # noqa: E402
import concourse.tile as tile  # noqa: E402
from concourse import mybir  # noqa: E402
from concourse.bass_utils import run_bass_kernel_spmd  # noqa: E402

import ml_dtypes  # noqa: E402

B, N, D, NCLS, U = 4, 1024, 256, 10, 140
NEG = -30000.0
F32 = mybir.dt.float32
F32R = mybir.dt.float32r
BF16 = mybir.dt.bfloat16
F16 = mybir.dt.float16
F8E5 = mybir.dt.float8e5
F8E4 = mybir.dt.float8e4
ALU = mybir.AluOpType
ACTF = mybir.ActivationFunctionType

# fpr (f32r) column layout
FPR_XTR = 0            # 2 x 1024 (x^T d-chunks, trunc22)
FPR_WALL = 2048        # 8 x 256 (w^T chunks: (q,k,v,a) x ft, trunc22)
FPR_ONES = 4096        # 1 col of 1.0
FPR_W = 4097
# fpc (f32) column layout
FPC_NROW = 0           # 512 iota row
FPC_QIOTA = 512        # 8 cols: p + 128*qt
FPC_ONES = 520         # 1 col of 1.0
FPC_BADD = 521         # 2 cols badd halves
FPC_ROW1 = 523         # 128 cols of 1.0 (row-broadcast stationary)
FPC_IDENT = 651        # 128x128 f32 identity (M transpose)
FPC_W = 779

# f16c (f16) column layout
F16_XB = 0             # 8 qt x 256 (x natural rows)
F16_WQT = 2048         # 2 ft x 256 (wq^T chunks)
F16_W = 2560


def build_nc(stage=6):
    nc = bacc.Bacc("TRN2", target_bir_lowering=False, debug=False, num_devices=8)

    fpr_d = nc.declare_dram_parameter("fpr", [128, FPR_W], F32R, isOutput=False)
    fpc_d = nc.declare_dram_parameter("fpc", [128, FPC_W], F32, isOutput=False)
    f16c_d = nc.declare_dram_parameter("f16c", [128, F16_W], F16, isOutput=False)
    am_d = nc.declare_dram_parameter("am_b", [128, 8 * N], F8E5, isOutput=False)
    id_d = nc.declare_dram_parameter("ident_b", [128, 128], F8E5, isOutput=False)
    ct_d = nc.declare_dram_parameter("ct_b", [128, 8 * N], F8E4, isOutput=False)
    wf_d = nc.declare_dram_parameter("wfin_r", [128, NCLS * N], BF16, isOutput=False)
    selc_d = nc.declare_dram_parameter("selc", [8, 8 * 128], F32, isOutput=False)
    out_d = nc.declare_dram_parameter("out20", [1, 2 * NCLS], F32, isOutput=True)

    mlin_d = nc.dram_tensor("m_lin", [N], F32)  # noqa: F841 (debug stages)

    def emit(tc):
        with (
            tc.tile_pool(name="const", bufs=1) as cpool,
            tc.tile_pool(name="big", bufs=1) as bpool,
            tc.tile_pool(name="scr", bufs=3) as spool,
            tc.tile_pool(name="small", bufs=1) as smpool,
        ):
            # ---- constant loads (packed) ----
            fpr = cpool.tile([128, FPR_W], F32R, name="fpr", tag="fpr")
            nc.sync.dma_start(fpr[:], fpr_d[:, :])
            ident = cpool.tile([128, 128], F8E5, name="ident", tag="ident")
            nc.sync.dma_start(ident[:], id_d[:, :])
            am = cpool.tile([128, 8 * N], F8E5, name="am", tag="am")
            ct = cpool.tile([128, 8 * N], F8E4, name="ct", tag="ct")
            # split so early qt tiles unblock sooner
            nc.sync.dma_start(am[:, 0: 4 * N], am_d[:, 0: 4 * N])
            nc.sync.dma_start(ct[:, 0: 4 * N], ct_d[:, 0: 4 * N])
            nc.sync.dma_start(am[:, 4 * N:], am_d[:, 4 * N:])
            nc.sync.dma_start(ct[:, 4 * N:], ct_d[:, 4 * N:])
            fpc = cpool.tile([128, FPC_W], F32, name="fpc", tag="fpc")
            nc.sync.dma_start(fpc[:], fpc_d[:, :])
            selc = cpool.tile([8, 8 * 128], F32, name="selc", tag="selc")
            nc.sync.dma_start(selc[:], selc_d[:, :])
            f16c = cpool.tile([128, F16_W], F16, name="f16c", tag="f16c")
            nc.scalar.dma_start(f16c[:], f16c_d[:, :])
            wf = cpool.tile([128, NCLS * N], BF16, name="wf", tag="wf")

            xtr = [fpr[:, FPR_XTR + i * N:FPR_XTR + (i + 1) * N]
                   for i in range(2)]
            wrb = {nm: [fpr[:, FPR_WALL + (2 * i + ft) * D:
                            FPR_WALL + (2 * i + ft + 1) * D]
                        for ft in range(2)]
                   for i, nm in enumerate(("q", "k", "v", "a"))}
            nrow = fpc[:, FPC_NROW:FPC_NROW + 512]
            qiota = fpc[:, FPC_QIOTA:FPC_QIOTA + 8]
            ones = fpc[:, FPC_ONES:FPC_ONES + 1]
            onesr = fpr[:, FPR_ONES:FPR_ONES + 1]
            badd = [fpc[:, FPC_BADD + i:FPC_BADD + i + 1] for i in range(2)]
            xb = [f16c[:, F16_XB + qt * D:F16_XB + (qt + 1) * D] for qt in range(8)]
            wqT = [f16c[:, F16_WQT + ft * D:F16_WQT + (ft + 1) * D] for ft in range(2)]

            # ---- phase B: K^T / Q^T projections ----
            ktT = [bpool.tile([128, N], F32R, name=f"ktT{i}", tag=f"ktT{i}") for i in range(2)]
            qtT = [bpool.tile([128, N], F32R, name=f"qtT{i}", tag=f"qtT{i}") for i in range(2)]
            ktT16 = [bpool.tile([128, N], F16, name=f"ktT16_{i}", tag=f"ktT16_{i}") for i in range(2)]
            # D+4 wide: cols D..D+3 = 1.0 (softmax denom via matmul; f32r
            # moving free size must be a multiple of 4)
            vna = [bpool.tile([128, D + 4], F32R, name=f"vna{i}", tag=f"vna{i}") for i in range(8)]
            vmean_row = smpool.tile([1, D], F32R, tag="vmean_row")
            maxacc = smpool.tile([128, 8], F32, tag="maxacc")
            sumacc = smpool.tile([128, 8], F32, tag="sumacc")
            with tc.tile_pool(name="psA", bufs=2, space="PSUM") as psA:
                for wt, dst in ((wrb["k"], ktT), (wrb["q"], qtT)):
                    for et in range(2):
                        ps = psA.tile([128, N], F32, tag="psA")
                        for nck in range(2):
                            for ft in range(2):
                                nc.tensor.matmul(
                                    ps[:, nck * 512:(nck + 1) * 512],
                                    wt[ft][:, et * 128:(et + 1) * 128],
                                    xtr[ft][:, nck * 512:(nck + 1) * 512],
                                    start=(ft == 0), stop=(ft == 1),
                                )
                        nc.scalar.copy(dst[et][:], ps[:])
            # f16 copies of K^T for phase E
            for et in range(2):
                nc.scalar.copy(ktT16[et][:], ktT[et][:])
            # Wfin load: dummy dep delays the 2.6MB DMA until the startup
            # burst has drained (fires ~when B finishes)
            nc.scalar.copy(wf[0:1, 0:1], ktT16[0][0:1, 0:1])
            nc.sync.dma_start(wf[:], wf_d[:, :])

            if stage == 1:
                dbg1 = smpool.tile([1, 20], F32, tag="dbg1")
                nc.scalar.copy(dbg1[:, 0:8], ktT[0][0:1, 0:8])
                nc.scalar.copy(dbg1[:, 8:16], qtT[1][0:1, 0:8])
                nc.sync.dma_start(out_d[:, :], dbg1[:])
                return

            # ---- phase C: QK (+mask on PE) + M reduces; V proj interleaved --
            with tc.tile_pool(name="psQK", bufs=3, space="PSUM") as psQK:
                for qt in range(8):
                    qk = psQK.tile([128, N], F32, tag="qk")
                    for kc in range(2):
                        for et in range(2):
                            nc.tensor.matmul(
                                qk[:, kc * 512:(kc + 1) * 512],
                                qtT[et][:, qt * 128:(qt + 1) * 128],
                                ktT[et][:, kc * 512:(kc + 1) * 512],
                                start=(et == 0), stop=False,
                            )
                        nc.tensor.matmul(
                            qk[:, kc * 512:(kc + 1) * 512], ident[:],
                            am[:, qt * N + kc * 512:qt * N + (kc + 1) * 512],
                            start=False, stop=True,
                        )
                    # f16 copy on the idle ACT engine frees the PSUM bank
                    # quickly; both DVE reduces read the SBUF f16 copy
                    qk16 = spool.tile([128, N], F16, tag="qk16")
                    nc.scalar.copy(qk16[:], qk[:])
                    # masked max (mask folded into qk) + sampled sum
                    # (qk*ct == (qk+am)*ct since ct=0 wherever am!=0)
                    nc.vector.tensor_reduce(
                        maxacc[:, qt:qt + 1], qk16[:], mybir.AxisListType.X,
                        ALU.max)
                    scr2 = spool.tile([128, N], F16, tag="scr2")
                    nc.vector.scalar_tensor_tensor(
                        scr2[:], qk16[:], 1.0 / N, ct[:, qt * N:(qt + 1) * N],
                        ALU.mult, ALU.mult, accum_out=sumacc[:, qt:qt + 1],
                    )


            m_sb = smpool.tile([128, 8], F32, tag="m_sb")
            nc.vector.tensor_sub(m_sb[:], maxacc[:], sumacc[:])

            if stage == 2:
                nc.sync.dma_start(out_d[:, 0:8], m_sb[0:1, :])
                return

            # ---- phase D: rank-based top-140; rank == compaction position --
            # M broadcast via PE transpose + rank-1 row broadcast (no DRAM
            # round trip); V / vmean / residual matmuls fill the rank window.
            identf = fpc[:, FPC_IDENT:FPC_IDENT + 128]
            row1 = fpc[0:1, FPC_ROW1:FPC_ROW1 + 128]
            mT = smpool.tile([8, 128], F32, tag="mT")
            rank = smpool.tile([128, 8], F32, tag="rank")
            e2 = [smpool.tile([128, U], F16, name=f"e2_{i}", tag=f"e2_{i}") for i in range(8)]
            er0 = smpool.tile([128, 512], F32R, tag="er0")
            er1 = smpool.tile([12, 512], F32R, tag="er1")
            fill_row = smpool.tile([1, 512], F32R, tag="fill_row")
            psR_ctx = tc.tile_pool(name="psR", bufs=1, space="PSUM")
            psR = psR_ctx.__enter__()
            psRt = [psR.tile([128, 512], F32, tag=f"psR{i}", name=f"psR{i}")
                    for i in range(2)]
            with tc.tile_pool(name="psM", bufs=1, space="PSUM") as psM, \
                 tc.tile_pool(name="psF", bufs=2, space="PSUM") as psF, \
                 tc.tile_pool(name="psVm", bufs=1, space="PSUM") as psVm:
                # PE fill work while DVE/gpsimd drain phase C: V chunks,
                # vmean, Wadd residual (all selection-independent)
                for qt in range(8):
                    psv = psF.tile([128, 512], F32, tag="psB2")
                    for ft in range(2):
                        nc.tensor.matmul(
                            psv[:, 0:D],
                            xtr[ft][:, qt * 128:(qt + 1) * 128],
                            wrb["v"][ft][:],
                            start=(ft == 0), stop=(ft == 1),
                        )
                    nc.scalar.copy(vna[qt][:, 0:D], psv[:, 0:D])
                vmean_ps = psVm.tile([1, D], F32, tag="vmean")
                for qt in range(8):
                    nc.tensor.matmul(
                        vmean_ps[:], onesr[:], vna[qt][:, 0:D],
                        start=(qt == 0), stop=(qt == 7),
                    )
                nc.scalar.mul(vmean_row[:], vmean_ps[:], 1.0 / N)
                for qt in range(8):
                    nc.scalar.copy(vna[qt][:, D:D + 4],
                                   fpc[:, FPC_ROW1:FPC_ROW1 + 4])
                for dtl in range(2):
                    ds = slice(dtl * 128, (dtl + 1) * 128)
                    for ft in range(2):
                        nc.tensor.matmul(psRt[dtl][:], wrb["a"][ft][:, ds],
                                         xtr[ft][:, 0:512],
                                         start=(ft == 0), stop=False)
                # M transpose + row broadcast (PE) -> rank (DVE) -> e2
                mTps = psM.tile([8, 128], F32, tag="mTps")
                nc.tensor.matmul(mTps[:], m_sb[:], identf[:],
                                 start=True, stop=True)
                nc.scalar.copy(mT[:], mTps[:])
                mbcP = psM.tile([128, N], F32, tag="mbcP")
                for qt in range(8):
                    nc.tensor.matmul(
                        mbcP[:, qt * 128:(qt + 1) * 128],
                        selc[:, qt * 128:(qt + 1) * 128],
                        mT[:], start=True, stop=True,
                    )
                for qt in range(8):
                    scrap = spool.tile([128, N], F32, tag="scr")
                    nc.vector.tensor_scalar(
                        scrap[:], mbcP[:], m_sb[:, qt:qt + 1], None, ALU.is_gt,
                        ALU.add, accum_out=rank[:, qt:qt + 1],
                    )
                # one-hot gather masks straight from the rank columns
                for qt in range(8):
                    nc.vector.tensor_scalar(
                        e2[qt][:], nrow[:, 0:U], rank[:, qt:qt + 1], None,
                        ALU.is_equal
                    )
            # scatter masks er[u, j] = (rank_n[j] == u): transpose rank and
            # row-broadcast on PE (no DRAM round trip), compare vs piota
            rkT = smpool.tile([8, 128], F32, tag="rkT")
            with tc.tile_pool(name="psN", bufs=1, space="PSUM") as psN:
                rkTps = psN.tile([8, 128], F32, tag="rkTps")
                nc.tensor.matmul(rkTps[:], rank[:], identf[:],
                                 start=True, stop=True)
                nc.scalar.copy(rkT[:], rkTps[:])
                rkbP = psN.tile([128, 512], F32, tag="rkbP")
                for jc in range(4):
                    nc.tensor.matmul(
                        rkbP[:, jc * 128:(jc + 1) * 128],
                        selc[:, jc * 128:(jc + 1) * 128],
                        rkT[:], start=True, stop=True,
                    )
                nc.vector.tensor_scalar(er0[:], rkbP[:], qiota[:, 0:1], None,
                                        ALU.is_equal)
                nc.vector.tensor_scalar(er1[:], rkbP[0:12, :],
                                        qiota[0:12, 1:2], None, ALU.is_equal)

            if stage == 4:
                dbg4 = smpool.tile([1, 20], F32, tag="dbg4")
                nc.scalar.copy(dbg4[:, 0:8], rank[0:1, :])
                nc.scalar.copy(dbg4[:, 8:16], rkT[0:1, 0:8])
                nc.sync.dma_start(out_d[:, :], dbg4[:])
                return

            # ---- phase E: X_red/Q_red gather, scores^T, softmax, attn@V ----
            xrT = [smpool.tile([128, U], F16, name=f"xrT{i}", tag=f"xrT{i}") for i in range(2)]
            qredT = [smpool.tile([128, U], F16, name=f"qredT{i}", tag=f"qredT{i}") for i in range(2)]
            expdT = [smpool.tile([128, U], F32R, name=f"expdT{i}", tag=f"expdT{i}") for i in range(8)]
            aug0 = smpool.tile([128, D], F32R, tag="aug0")
            aug1 = smpool.tile([12, D], F32R, tag="aug1")
            with tc.tile_pool(name="psC", bufs=2, space="PSUM") as psC, \
                 tc.tile_pool(name="psCt", bufs=1, space="PSUM") as psCt, \
                 tc.tile_pool(name="psE", bufs=2, space="PSUM") as psE:
                for ft in range(2):
                    ps = psC.tile([128, U], F32, tag="psC")
                    for qt in range(8):
                        nc.tensor.matmul(
                            ps[:],
                            xb[qt][:, ft * 128:(ft + 1) * 128],
                            e2[qt][:],
                            start=(qt == 0), stop=(qt == 7),
                        )
                    nc.scalar.copy(xrT[ft][:], ps[:])
                for ec in range(2):
                    ps = psC.tile([128, U], F32, tag="psC")
                    for ft in range(2):
                        nc.tensor.matmul(
                            ps[:],
                            wqT[ft][:, ec * 128:(ec + 1) * 128],
                            xrT[ft][:],
                            start=(ft == 0), stop=(ft == 1),
                        )
                    nc.scalar.copy(qredT[ec][:], ps[:])
                for kt in range(8):
                    ps = psC.tile([128, U], F32, tag="psC")
                    for et in range(2):
                        nc.tensor.matmul(
                            ps[:], ktT16[et][:, kt * 128:(kt + 1) * 128], qredT[et][:],
                            start=(et == 0), stop=(et == 1),
                        )
                    nc.scalar.activation(
                        expdT[kt][:], ps[:], ACTF.Exp, scale=1.0 / math.sqrt(D)
                    )
                if stage == 5:
                    dbg = smpool.tile([1, 20], F32, tag="dbg5")
                    nc.scalar.copy(dbg[:], expdT[0][0:1, 0:20].bitcast(F32))
                    nc.sync.dma_start(out_d[:, :], dbg[:])
                    return
                for sc, (p0, np_) in enumerate(((0, 128), (128, 12))):
                    ps = psE.tile([np_, D + 4], F32, tag="psE")
                    for kt in range(8):
                        nc.tensor.matmul(
                            ps[:], expdT[kt][:, p0:p0 + np_], vna[kt][:],
                            start=(kt == 0), stop=(kt == 7),
                        )
                    rc = smpool.tile([np_, 1], F32, tag=f"recip{sc}")
                    nc.vector.reciprocal(rc[:], ps[:, D:D + 1])
                    dst = aug0[:] if sc == 0 else aug1[:]
                    nc.vector.tensor_scalar(
                        dst, ps[:, 0:D], rc[:], None, ALU.mult
                    )
                # vmean fill weights (er-dependent, off the E critical path)
                cntp = psCt.tile([1, 512], F32, tag="cnt")
                nc.tensor.matmul(cntp[:], onesr[:], er0[:], start=True, stop=False)
                nc.tensor.matmul(cntp[:], onesr[0:12, :], er1[:],
                                 start=False, stop=True)
                nc.scalar.activation(fill_row[:], cntp[:], ACTF.Copy,
                                     bias=1.0, scale=-1.0)

            # ---- phase F: scatter + residual in PSUM + fused class dots ----
            facc = smpool.tile([128, 2 * NCLS], F32, tag="facc")
            ctxh = smpool.tile([128, N], BF16, name="ctxh", tag="ctxh")
            if True:
                for dtl in range(2):
                    ds = slice(dtl * 128, (dtl + 1) * 128)
                    ps = psRt[dtl]
                    nc.tensor.matmul(ps[:], aug0[:, ds], er0[:],
                                     start=False, stop=False)
                    nc.tensor.matmul(ps[:], aug1[:, ds], er1[:],
                                     start=False, stop=False)
                    nc.tensor.matmul(ps[:], vmean_row[0:1, ds], fill_row[0:1, :],
                                     start=False, stop=True)
                    nc.scalar.activation(ctxh[:, dtl * 512:(dtl + 1) * 512], ps[:],
                                         ACTF.Identity, bias=badd[dtl][:], scale=1.0)
                    for cls in range(NCLS):
                        scr = spool.tile([128, 512], BF16, tag="scrf")
                        nc.vector.scalar_tensor_tensor(
                            scr[:], ctxh[:, dtl * 512:(dtl + 1) * 512], 1.0,
                            wf[:, cls * N + dtl * 512:cls * N + (dtl + 1) * 512],
                            ALU.mult, ALU.mult,
                            accum_out=facc[:, 2 * cls + dtl:2 * cls + dtl + 1],
                        )
            psR_ctx.__exit__(None, None, None)
            with tc.tile_pool(name="psO", bufs=1, space="PSUM") as psO:
                o = psO.tile([1, 2 * NCLS], F32, tag="o")
                nc.tensor.matmul(o[:], ones[:], facc[:],
                                 start=True, stop=True)
                osb = smpool.tile([1, 2 * NCLS], F32, tag="osb")
                nc.scalar.copy(osb[:], o[:])
                nc.sync.dma_start(out_d[:, :], osb[:])

    with tile.TileContext(nc) as tc:
        emit(tc)
    nc.compile()
    return nc


_NC_CACHE = {}


def get_nc(stage=6):
    if stage not in _NC_CACHE:
        _NC_CACHE[stage] = build_nc(stage)
    return _NC_CACHE[stage]


def host_prep(inputs):
    """Build per-core input maps from the full problem inputs."""
    x = np.asarray(inputs["input_embedding"], np.float32)        # [B, N, D]
    wq = np.asarray(inputs["Wq"], np.float32)
    wk = np.asarray(inputs["Wk"], np.float32)
    wv = np.asarray(inputs["Wv"], np.float32)
    wa = np.asarray(inputs["Wadd"], np.float32)
    badd = np.asarray(inputs["badd"], np.float32)
    wfin = np.asarray(inputs["Wfin"], np.float32)                # [10, N*D]
    idx = np.asarray(inputs["index_sample"]).astype(np.int64)    # [N, U]

    cnt = np.zeros((N, N), np.float32)
    np.add.at(cnt, (np.arange(N)[:, None], idx), 1.0)

    # The device program always dots ctx[:, 0:512] against its Wfin shard.
    # Core half h=1 gets the n-axis halves swapped on every n-indexed input
    # (the pipeline is equivariant under a joint permutation of X rows,
    # mask rows+cols, and Wfin columns), so "columns 0:512" is its half.
    perms = [np.arange(N), np.concatenate([np.arange(512, N), np.arange(512)])]
    am_h, ct_h = [], []
    for p in perms:
        cp = cnt[p][:, p]
        amf = np.where(cp > 0, 0.0, -28672.0).astype(ml_dtypes.float8_e5m2)
        ctf = cp.astype(ml_dtypes.float8_e4m3fn)
        # [N, N] rows -> [128, 8*N] qt-major
        am_h.append(np.ascontiguousarray(
            amf.reshape(8, 128, N).transpose(1, 0, 2).reshape(128, 8 * N)))
        ct_h.append(np.ascontiguousarray(
            ctf.reshape(8, 128, N).transpose(1, 0, 2).reshape(128, 8 * N)))

    # Wfin[c, n*256+d] -> [10, d, n] -> halves -> [128, 10*1024]
    wr = wfin.reshape(NCLS, N, D).transpose(0, 2, 1)             # [10, 256, 1024]
    wr_h = []
    for h in range(2):
        w = wr[:, :, h * 512:(h + 1) * 512].reshape(NCLS, 2, 128, 512)
        w = w.transpose(2, 0, 1, 3).reshape(128, NCLS * N)       # [128, 10*1024]
        wr_h.append(np.ascontiguousarray(w).astype(ml_dtypes.bfloat16))

    def trunc22(a):
        b = np.ascontiguousarray(a, np.float32)
        return (b.view(np.uint32) & np.uint32(0xFFFFFC00)).view(np.float32)

    w_all = np.stack([trunc22(w.T).reshape(2, 128, D) for w in (wq, wk, wv, wa)])
    # fpr skeleton (x^T filled per core half) + fpc misc block
    fpr0 = np.zeros((128, FPR_W), np.float32)
    fpr0[:, FPR_WALL:FPR_WALL + 8 * D] = (
        w_all.transpose(2, 0, 1, 3).reshape(128, 8 * D))
    fpr0[:, FPR_ONES] = 1.0
    fpc0 = np.zeros((128, FPC_W), np.float32)
    fpc0[:, FPC_NROW:FPC_NROW + 512] = np.arange(512, dtype=np.float32)[None, :]
    fpc0[:, FPC_QIOTA:FPC_QIOTA + 8] = (
        np.arange(128, dtype=np.float32)[:, None]
        + 128.0 * np.arange(8, dtype=np.float32)[None, :])
    fpc0[:, FPC_ONES] = 1.0
    fpc0[:, FPC_BADD] = badd[0:128]
    fpc0[:, FPC_BADD + 1] = badd[128:256]
    fpc0[:, FPC_ROW1:FPC_ROW1 + 128] = 1.0
    fpc0[:, FPC_IDENT:FPC_IDENT + 128] = np.eye(128, dtype=np.float32)
    selc0 = np.zeros((8, 8 * 128), np.float32)
    for c8 in range(8):
        selc0[c8, c8 * 128:(c8 + 1) * 128] = 1.0

    wqT16 = trunc22(wq.T).reshape(2, 128, D).transpose(1, 0, 2).reshape(128, 2 * D)

    consts = {
        "fpc": fpc0,
        "ident_b": np.eye(128, dtype=np.float32).astype(ml_dtypes.float8_e5m2),
        "selc": selc0,
    }

    in_maps = []
    core_cache = {}
    for c in range(8):
        b, h = c // 2, c % 2
        m = dict(consts)
        if (b, h) not in core_cache:
            xp = np.ascontiguousarray(x[b][perms[h]])            # [N, D]
            xtc = trunc22(xp.T)                                  # [D, N]
            fpr = fpr0.copy()
            fpr[:, FPR_XTR:FPR_XTR + 2 * N] = (
                xtc.reshape(2, 128, N).transpose(1, 0, 2).reshape(128, 2 * N))
            f16cb = np.zeros((128, F16_W), np.float16)
            f16cb[:, F16_XB:F16_XB + 8 * D] = (
                xp.reshape(8, 128, D).transpose(1, 0, 2).reshape(128, 8 * D)
                .astype(np.float16))
            f16cb[:, F16_WQT:F16_WQT + 2 * D] = wqT16.astype(np.float16)
            core_cache[(b, h)] = (fpr, f16cb)
        m["fpr"], m["f16c"] = core_cache[(b, h)]
        m["am_b"] = am_h[h]
        m["ct_b"] = ct_h[h]
        m["wfin_r"] = wr_h[h]
        in_maps.append(m)
    return in_maps


def host_combine(results, inputs):
    bfin = np.asarray(inputs["bfin"], np.float32)
    out = np.zeros((B, NCLS), np.float32)
    for c in range(8):
        b = c // 2
        o = results[c]["out20"].reshape(2 * NCLS)
        out[b] += o[0::2] + o[1::2]
    return out + bfin[None, :]


def kernel(**inputs):
    nc = get_nc()
    in_maps = host_prep(inputs)
    res = run_bass_kernel_spmd(nc, in_maps, core_ids=list(range(8)))
    return host_combine(res.results, inputs)
